# revision 7
# baseline (speedup 1.0000x reference)
"""2-layer GCN + JumpingKnowledge(cat) + Linear on 8 Trainium2 NeuronCores.

v2 strategy (aggregate-first, bf16, graph-parallel, nodes 6250/core):
  - GCN identity  relu(D^-1/2 (A+I) D^-1/2 X W + b) = relu(((A+I)_norm X) W + b):
    aggregate FIRST, transform after.  Layer 1 therefore gathers straight
    from a host-precomputed table  x~ = dinv * x  (bf16, replicated input):
    no phase matmul, no AllGather before the first gather - the gpsimd
    gather stream starts at t~0.
  - Aggregation per destination tile (128 dsts): dma_gather pulls the
    unique source rows (bf16, 256B) from the table; TensorE matmul chain
    psum[feat,dst] += M_chunk^T @ S_chunk with S (host-built, bf16)
    carrying dinv[dst] and edge multiplicity.  Self-loops via diag(dinv)
    against the local row-major x~ tiles (no gather).
  - Post-aggregate per tile: Y -> sbuf,  x1T = relu(W^T Y + b) (feat-major,
    feeds final linear),  PE-transpose + dinv scale -> x~1 (row-major,
    bf16) -> DRAM table rows -> AllGather (A-half triggered mid-layer-1 so
    the collective hides under the gather stream; B-half at layer-1 end,
    hidden under layer-2's A-pass).
  - Gathers are per (tile, half-table); index padding uses row 0 (S has
    zero coefficients there, so padded rows contribute nothing).
  - Final: out = x1 @ lin_W[:D] + x2 @ lin_W[D:] + lin_b per tile, fused
    into layer-2's second pass.

dma_gather needs int16 indices (<32768 rows), so tables are split in two
blocks: each core contributes rows [0,3200) to table A and [3200,6272) to
table B.
"""
import numpy as np
import ml_dtypes

import concourse.bass as bass
import concourse.bacc as bacc
import concourse.mybir as mybir
import concourse.tile as tile
from concourse._compat import get_trn_type
from concourse.bass_utils import run_bass_kernel_spmd
from concourse.library_config import mlp
from concourse.masks import make_identity

P = 128
N_CORES = 8

f32 = mybir.dt.float32
bf16 = mybir.dt.bfloat16
i16 = mybir.dt.int16


def _preprocess(x, edge_index, W1, b1, W2, b2, lin_W, lin_b):
    """Host-side: shard, block-split, dedup, build gather indices (-1
    padded), S matrices, the dinv-scaled bf16 x table, and all in_maps."""
    N, D = x.shape
    assert D == P
    E = edge_index.shape[1]
    C = N_CORES
    NPC = (N + C - 1) // C
    assert NPC * C == N, "node count must split evenly across cores"
    NPAD = ((NPC + P - 1) // P) * P
    T = NPAD // P
    TA = (T + 1) // 2
    TB = T - TA
    BLKA, BLKB = TA * P, TB * P
    FULLA, FULLB = C * BLKA, C * BLKB
    assert FULLA <= 32768 and FULLB <= 32768

    src = edge_index[0].astype(np.int64)
    dst = edge_index[1].astype(np.int64)

    deg = np.bincount(dst, minlength=N).astype(np.float32) + 1.0
    dinv = (1.0 / np.sqrt(deg)).astype(np.float32)

    off = src % NPC
    blk = (off >= BLKA).astype(np.int64)
    row_in_blk = np.where(blk == 0, (src // NPC) * BLKA + off,
                          (src // NPC) * BLKB + (off - BLKA))
    core = dst // NPC
    dloc = dst % NPC
    tl = dloc // P
    dcol = dloc % P
    vals = dinv[dst]

    gkey = (core * T + tl) * 2 + blk
    order = np.lexsort((row_in_blk, gkey))
    gkey_s = gkey[order]
    rows_s = row_in_blk[order]
    dcol_s = dcol[order]
    vals_s = vals[order]
    n_groups = C * T * 2
    bounds = np.searchsorted(gkey_s, np.arange(n_groups + 1))

    uniq = [None] * n_groups
    invs = [None] * n_groups
    for g in range(n_groups):
        lo, hi = bounds[g], bounds[g + 1]
        r = rows_s[lo:hi]
        u = np.unique(r)
        uniq[g] = u
        invs[g] = np.searchsorted(u, r)

    # SPMD-uniform schedule: max over cores per (tile, half)
    nuniq = np.zeros((C, T, 2), np.int64)
    for g in range(n_groups):
        c, rem = divmod(g, T * 2)
        t, h = divmod(rem, 2)
        nuniq[c, t, h] = len(uniq[g])
    reg = np.maximum(nuniq.max(axis=0), 1)  # [T, 2]
    sched = (reg + P - 1) // P              # [T, 2] chunks (128-multiples)
    SCH = int(sched.sum())

    # chunk layout: per tile, half A then half B
    chunk_off = np.zeros((T, 2), np.int64)
    acc = 0
    for t in range(T):
        for h in range(2):
            chunk_off[t, h] = acc
            acc += int(sched[t, h])
    assert acc == SCH

    # global dinv-scaled x table in block layout (replicated to all cores)
    xs = (dinv[:, None] * x).astype(np.float32)
    xpad = np.zeros((C, NPAD, P), np.float32)
    for c in range(C):
        xpad[c, :NPC] = xs[c * NPC : (c + 1) * NPC]
    xA = np.ascontiguousarray(xpad[:, :BLKA].reshape(FULLA, P)).astype(ml_dtypes.bfloat16)
    xB = np.ascontiguousarray(xpad[:, BLKA:].reshape(FULLB, P)).astype(ml_dtypes.bfloat16)

    common = {
        "xA": xA, "xB": xB,
        "w1": W1.astype(ml_dtypes.bfloat16),
        "w2": W2.astype(ml_dtypes.bfloat16),
        "lin1": np.ascontiguousarray(lin_W[:D]).astype(ml_dtypes.bfloat16),
        "lin2": np.ascontiguousarray(lin_W[D:]).astype(ml_dtypes.bfloat16),
        "b1": b1[:, None].astype(np.float32),
        "b2": b2[:, None].astype(np.float32),
        "linb": np.tile(lin_b, (P, 1)).astype(np.float32),
    }

    in_maps = []
    for c in range(C):
        idx_cols = np.full((P, SCH * 8), -1, np.int16)
        cap = E // C + 4 * SCH * P + E // 16
        flat_rows = np.zeros(cap, np.int64)
        flat_dcol = np.zeros(cap, np.int64)
        flat_vals = np.zeros(cap, np.float32)
        n_e = 0
        for t in range(T):
            for h in range(2):
                g = (c * T + t) * 2 + h
                nch = int(sched[t, h])
                K = nch * P
                u = uniq[g]
                u_pad = np.zeros(K, np.int64)
                u_pad[: len(u)] = u
                wrapped = u_pad.astype(np.int16).reshape(nch * 8, 16).T
                co = int(chunk_off[t, h])
                idx_cols[:, co * 8 : (co + nch) * 8] = np.tile(wrapped, (8, 1))
                lo, hi = bounds[g], bounds[g + 1]
                ne = hi - lo
                flat_rows[n_e : n_e + ne] = co * P + invs[g]
                flat_dcol[n_e : n_e + ne] = dcol_s[lo:hi]
                flat_vals[n_e : n_e + ne] = vals_s[lo:hi]
                n_e += ne
        flat = flat_rows[:n_e] * P + flat_dcol[:n_e]
        s_core = np.bincount(flat, weights=flat_vals[:n_e], minlength=SCH * P * P)
        s_core = s_core.astype(np.float32).reshape(SCH, P, P)
        smat = np.ascontiguousarray(s_core.transpose(1, 0, 2)).reshape(P, SCH * P)

        # local x~ tiles, row-major [node-in-tile (part), tile, feat]
        xloc = np.ascontiguousarray(
            xpad[c].reshape(T, P, P).transpose(1, 0, 2)
        ).reshape(P, T * P)

        dv = np.zeros(NPAD, np.float32)
        dv[:NPC] = dinv[c * NPC : (c + 1) * NPC]
        dinv_tiles = np.ascontiguousarray(dv.reshape(T, P).T)  # [P, T]

        in_maps.append(dict(common) | {
            "xloc": xloc.astype(ml_dtypes.bfloat16),
            "dinv": dinv_tiles,
            "idx": idx_cols,
            "smat": smat.astype(ml_dtypes.bfloat16),
        })

    plan = {
        "N": N, "D": D, "E": E, "C": C, "NPC": NPC, "NPAD": NPAD, "T": T,
        "TA": TA, "TB": TB, "BLKA": BLKA, "BLKB": BLKB,
        "FULLA": FULLA, "FULLB": FULLB, "SCH": SCH,
        "sched": sched, "chunk_off": chunk_off,
    }
    return plan, in_maps


def _build(plan):
    T, TA, TB = plan["T"], plan["TA"], plan["TB"]
    NPAD = plan["NPAD"]
    BLKA, BLKB = plan["BLKA"], plan["BLKB"]
    FULLA, FULLB = plan["FULLA"], plan["FULLB"]
    SCH = plan["SCH"]
    sched = plan["sched"]
    chunk_off = plan["chunk_off"]
    maxch = int(sched.max())

    nc = bacc.Bacc(
        get_trn_type() or "TRN2",
        target_bir_lowering=False,
        debug=False,
        num_devices=N_CORES,
    )
    xA_in = nc.dram_tensor("xA", [FULLA, P], bf16, kind="ExternalInput").ap()
    xB_in = nc.dram_tensor("xB", [FULLB, P], bf16, kind="ExternalInput").ap()
    xloc_in = nc.dram_tensor("xloc", [P, T * P], bf16, kind="ExternalInput").ap()
    w1_in = nc.dram_tensor("w1", [P, P], bf16, kind="ExternalInput").ap()
    w2_in = nc.dram_tensor("w2", [P, P], bf16, kind="ExternalInput").ap()
    lin1_in = nc.dram_tensor("lin1", [P, P], bf16, kind="ExternalInput").ap()
    lin2_in = nc.dram_tensor("lin2", [P, P], bf16, kind="ExternalInput").ap()
    b1_in = nc.dram_tensor("b1", [P, 1], f32, kind="ExternalInput").ap()
    b2_in = nc.dram_tensor("b2", [P, 1], f32, kind="ExternalInput").ap()
    linb_in = nc.dram_tensor("linb", [P, P], f32, kind="ExternalInput").ap()
    dinv_in = nc.dram_tensor("dinv", [P, T], f32, kind="ExternalInput").ap()
    idx_in = nc.dram_tensor("idx", [P, SCH * 8], i16, kind="ExternalInput").ap()
    smat_in = nc.dram_tensor("smat", [P, SCH * P], bf16, kind="ExternalInput").ap()
    out_ap = nc.dram_tensor("out", [NPAD, P], f32, kind="ExternalOutput").ap()
    out_v = out_ap.rearrange("(t p) f -> p t f", p=P)

    nc.gpsimd.load_library(mlp)

    with tile.TileContext(nc) as tc:
        with (
            tc.tile_pool(name="dram", bufs=1, space="DRAM") as dram,
            tc.tile_pool(name="consts", bufs=1) as consts,
            tc.tile_pool(name="acts", bufs=1) as acts,
            tc.tile_pool(name="msg", bufs=6) as msgp,
            tc.tile_pool(name="smatp", bufs=6) as smatp,
            tc.tile_pool(name="diag", bufs=3) as diagp,
            tc.tile_pool(name="ysb", bufs=3) as ysbp,
            tc.tile_pool(name="otile", bufs=3) as otilep,
            tc.tile_pool(name="ps_y", bufs=3, space="PSUM") as psy,
            tc.tile_pool(name="ps_tf", bufs=4, space="PSUM") as pstf,
        ):
            w1 = consts.tile([P, P], bf16, tag="w1")
            nc.sync.dma_start(w1[:], w1_in[:])
            w2 = consts.tile([P, P], bf16, tag="w2")
            nc.sync.dma_start(w2[:], w2_in[:])
            lin1 = consts.tile([P, P], bf16, tag="lin1")
            nc.sync.dma_start(lin1[:], lin1_in[:])
            lin2 = consts.tile([P, P], bf16, tag="lin2")
            nc.sync.dma_start(lin2[:], lin2_in[:])
            b1 = consts.tile([P, 1], f32, tag="b1")
            nc.sync.dma_start(b1[:], b1_in[:])
            b2 = consts.tile([P, 1], f32, tag="b2")
            nc.sync.dma_start(b2[:], b2_in[:])
            linb = consts.tile([P, P], f32, tag="linb")
            nc.sync.dma_start(linb[:], linb_in[:])
            dinv = consts.tile([P, T], f32, tag="dinv")
            nc.sync.dma_start(dinv[:], dinv_in[:])
            idx_sb = consts.tile([P, SCH * 8], i16, tag="idx")
            nc.sync.dma_start(idx_sb[:], idx_in[:])
            ident = consts.tile([P, P], bf16, tag="ident")
            make_identity(nc, ident[:])

            xloc = acts.tile([P, T * P], bf16, tag="xloc")
            nc.sync.dma_start(xloc[:], xloc_in[:])
            x1loc = acts.tile([P, T * P], bf16, tag="x1loc")   # dinv-scaled x1
            x1T = acts.tile([P, NPAD], bf16, tag="x1T")        # feat-major
            x2T = acts.tile([P, NPAD], bf16, tag="x2T")
            partial = acts.tile([P, NPAD], f32, tag="partial")

            # memset msg buffers once so trailing-trimmed (skipped) gather
            # chunks read as finite values (x * 0 = 0 in the S matmul)
            for _ in range(6):
                m0 = msgp.tile([P, maxch, P], bf16, tag="msg", name="msg")
                nc.vector.memset(m0[:, :, :], 0)

            g_loc = [None, None]
            g_full = [None, None]
            for h, (blkrows, fullrows) in enumerate([(BLKA, FULLA), (BLKB, FULLB)]):
                g_loc[h] = dram.tile([blkrows, P], bf16, tag=f"gloc{h}",
                                     name=f"gloc{h}")
                g_full[h] = dram.tile([fullrows, P], bf16, tag=f"gfull{h}",
                                      name=f"gfull{h}")

            tabA = [xA_in, g_full[0]]
            tabB = [xB_in, g_full[1]]

            def gather_th(layer, t, h):
                """Issue dma_gather + S load for (tile t, half h); return
                (msg, s_sb, nch)."""
                nch = int(sched[t, h])
                K = nch * P
                co = int(chunk_off[t, h])
                msg = msgp.tile([P, maxch, P], bf16, tag="msg", name="msg")
                s_sb = smatp.tile([P, maxch * P], bf16, tag="smat", name="ssb")
                nc.sync.dma_start(
                    s_sb[:, 0 : nch * P], smat_in[:, co * P : (co + nch) * P]
                )
                table = tabA[layer] if h == 0 else tabB[layer]
                nc.gpsimd.dma_gather(
                    msg[:, 0:nch, :], table[:], idx_sb[:, co * 8 : (co + nch) * 8],
                    K, K, P, single_packet=False,
                )
                return msg, s_sb, nch

            def agg_tile(ps, pieces, t, src_loc, add_partial):
                """S-matmul chain + self-loop (+partial) into psum ps."""
                first = True
                for msg, s_sb, nch in pieces:
                    for j in range(nch):
                        nc.tensor.matmul(
                            ps[:], lhsT=msg[:, j, :], rhs=s_sb[:, bass.ts(j, P)],
                            start=first, stop=False,
                        )
                        first = False
                diag = diagp.tile([P, P], bf16, tag="diag", name="diag")
                nc.vector.tensor_scalar(
                    diag[:], ident[:], dinv[:, t : t + 1], None,
                    mybir.AluOpType.mult,
                )
                nc.tensor.matmul(
                    ps[:], lhsT=src_loc[:, bass.ts(t, P)], rhs=diag[:],
                    start=first, stop=True,
                )
                if add_partial:
                    nc.vector.tensor_tensor(
                        out=ps[:], in0=ps[:], in1=partial[:, bass.ts(t, P)],
                        op=mybir.AluOpType.add,
                    )

            def transform_tile(ps, t, w_tile, bias, xT_out, make_table):
                """Y(psum) -> xT (feat-major) and optionally the dinv-scaled
                row-major table tile + DRAM write."""
                ysb = ysbp.tile([P, P], bf16, tag="ysb", name="ysb")
                nc.vector.tensor_copy(out=ysb[:], in_=ps[:])
                ps2 = pstf.tile([P, P], f32, tag="ps_tf", name="pstf")
                nc.tensor.matmul(ps2[:], lhsT=w_tile[:], rhs=ysb[:],
                                 start=True, stop=True)
                nc.scalar.activation(
                    xT_out[:, bass.ts(t, P)], ps2[:],
                    mybir.ActivationFunctionType.Relu, bias=bias[:],
                )
                if make_table:
                    psT = pstf.tile([P, P], f32, tag="ps_tf", name="pstf")
                    nc.tensor.matmul(psT[:], lhsT=xT_out[:, bass.ts(t, P)],
                                     rhs=ident[:], start=True, stop=True)
                    nc.vector.tensor_scalar(
                        x1loc[:, bass.ts(t, P)], psT[:],
                        dinv[:, t : t + 1], None, mybir.AluOpType.mult,
                    )

            def allgather(h):
                lo = 0 if h == 0 else BLKA
                hi = BLKA if h == 0 else NPAD
                nc.sync.dma_start(
                    g_loc[h][:].rearrange("(tt p) f -> p tt f", p=P),
                    x1loc[:, lo:hi].rearrange("p (tt f) -> p tt f", f=P),
                )
                nc.gpsimd.collective_compute(
                    "AllGather",
                    mybir.AluOpType.bypass,
                    replica_groups=[list(range(N_CORES))],
                    ins=[g_loc[h].opt()],
                    outs=[g_full[h].opt()],
                )

            # ---------------- layer 1: single pass over tiles ----------------
            for t in range(T):
                pieces = [gather_th(0, t, 0), gather_th(0, t, 1)]
                ps = psy.tile([P, P], f32, tag="ps_y", name="psy")
                agg_tile(ps, pieces, t, xloc, add_partial=False)
                transform_tile(ps, t, w1, b1, x1T, make_table=True)
                if t == TA:  # tiles 0..TA-1 (A rows) written -> start AllGather
                    allgather(0)
            allgather(1)

            # ---------------- layer 2: pass 1 (A half) -> partial ------------
            for t in range(T):
                msg, s_sb, nch = gather_th(1, t, 0)
                ps = psy.tile([P, P], f32, tag="ps_y", name="psy")
                for j in range(nch):
                    nc.tensor.matmul(
                        ps[:], lhsT=msg[:, j, :], rhs=s_sb[:, bass.ts(j, P)],
                        start=(j == 0), stop=(j == nch - 1),
                    )
                nc.vector.tensor_copy(out=partial[:, bass.ts(t, P)], in_=ps[:])

            # ---------------- layer 2: pass 2 (B half) + final ---------------
            for t in range(T):
                pieces = [gather_th(1, t, 1)]
                ps = psy.tile([P, P], f32, tag="ps_y", name="psy")
                agg_tile(ps, pieces, t, x1loc, add_partial=True)
                transform_tile(ps, t, w2, b2, x2T, make_table=False)
                # final: out_t = x1_t @ lin1 + x2_t @ lin2 + lin_b
                ps3 = pstf.tile([P, P], f32, tag="ps_tf", name="pstf")
                nc.tensor.matmul(ps3[:], lhsT=x1T[:, bass.ts(t, P)],
                                 rhs=lin1[:], start=True, stop=False)
                nc.tensor.matmul(ps3[:], lhsT=x2T[:, bass.ts(t, P)],
                                 rhs=lin2[:], start=False, stop=True)
                ot = otilep.tile([P, P], f32, tag="otile", name="otile")
                nc.vector.tensor_tensor(
                    out=ot[:], in0=ps3[:], in1=linb[:], op=mybir.AluOpType.add
                )
                nc.sync.dma_start(out_v[:, t, :], ot[:])

    nc.compile()
    return nc


def kernel(x, edge_index, W1, b1, W2, b2, lin_W, lin_b):
    x = np.asarray(x, np.float32)
    edge_index = np.asarray(edge_index)
    W1 = np.asarray(W1, np.float32)
    W2 = np.asarray(W2, np.float32)
    b1 = np.asarray(b1, np.float32)
    b2 = np.asarray(b2, np.float32)
    lin_W = np.asarray(lin_W, np.float32)
    lin_b = np.asarray(lin_b, np.float32)

    plan, in_maps = _preprocess(x, edge_index, W1, b1, W2, b2, lin_W, lin_b)
    nc = _build(plan)

    N, D, C, NPC = plan["N"], plan["D"], plan["C"], plan["NPC"]
    last_err = None
    for _attempt in range(3):
        try:
            res = run_bass_kernel_spmd(nc, in_maps, list(range(C)))
            break
        except Exception as e:  # transient NRT device wedges happen
            last_err = e
    else:
        raise last_err

    out = np.empty((N, D), np.float32)
    for c in range(C):
        out[c * NPC : (c + 1) * NPC] = res.results[c]["out"][:NPC]
    return out


# revision 8
# speedup vs baseline: 1.0030x; 1.0030x over previous
"""2-layer GCN + JumpingKnowledge(cat) + Linear on 8 Trainium2 NeuronCores.

v2 strategy (aggregate-first, bf16, graph-parallel, nodes 6250/core):
  - GCN identity  relu(D^-1/2 (A+I) D^-1/2 X W + b) = relu(((A+I)_norm X) W + b):
    aggregate FIRST, transform after.  Layer 1 therefore gathers straight
    from a host-precomputed table  x~ = dinv * x  (bf16, replicated input):
    no phase matmul, no AllGather before the first gather - the gpsimd
    gather stream starts at t~0.
  - Aggregation per destination tile (128 dsts): dma_gather pulls the
    unique source rows (bf16, 256B) from the table; TensorE matmul chain
    psum[feat,dst] += M_chunk^T @ S_chunk with S (host-built, bf16)
    carrying dinv[dst] and edge multiplicity.  Self-loops via diag(dinv)
    against the local row-major x~ tiles (no gather).
  - Post-aggregate per tile: Y -> sbuf,  x1T = relu(W^T Y + b) (feat-major,
    feeds final linear),  PE-transpose + dinv scale -> x~1 (row-major,
    bf16) -> DRAM table rows -> AllGather (A-half triggered mid-layer-1 so
    the collective hides under the gather stream; B-half at layer-1 end,
    hidden under layer-2's A-pass).
  - Gathers are per (tile, half-table); index padding uses row 0 (S has
    zero coefficients there, so padded rows contribute nothing).
  - Final: out = x1 @ lin_W[:D] + x2 @ lin_W[D:] + lin_b per tile, fused
    into layer-2's second pass.

dma_gather needs int16 indices (<32768 rows), so tables are split in two
blocks: each core contributes rows [0,3200) to table A and [3200,6272) to
table B.
"""
import numpy as np
import ml_dtypes

import concourse.bass as bass
import concourse.bacc as bacc
import concourse.mybir as mybir
import concourse.tile as tile
from concourse._compat import get_trn_type
from concourse.bass_utils import run_bass_kernel_spmd
from concourse.library_config import mlp

P = 128
N_CORES = 8

f32 = mybir.dt.float32
bf16 = mybir.dt.bfloat16
i16 = mybir.dt.int16


def _preprocess(x, edge_index, W1, b1, W2, b2, lin_W, lin_b):
    """Host-side: shard, block-split, dedup, build gather indices (-1
    padded), S matrices, the dinv-scaled bf16 x table, and all in_maps."""
    N, D = x.shape
    assert D == P
    E = edge_index.shape[1]
    C = N_CORES
    NPC = (N + C - 1) // C
    assert NPC * C == N, "node count must split evenly across cores"
    NPAD = ((NPC + P - 1) // P) * P
    T = NPAD // P
    TA = (T + 1) // 2
    TB = T - TA
    BLKA, BLKB = TA * P, TB * P
    FULLA, FULLB = C * BLKA, C * BLKB
    assert FULLA <= 32768 and FULLB <= 32768

    src = edge_index[0].astype(np.int64)
    dst = edge_index[1].astype(np.int64)

    deg = np.bincount(dst, minlength=N).astype(np.float32) + 1.0
    dinv = (1.0 / np.sqrt(deg)).astype(np.float32)

    off = src % NPC
    blk = (off >= BLKA).astype(np.int64)
    row_in_blk = np.where(blk == 0, (src // NPC) * BLKA + off,
                          (src // NPC) * BLKB + (off - BLKA))
    core = dst // NPC
    dloc = dst % NPC
    tl = dloc // P
    dcol = dloc % P
    vals = dinv[dst]

    gkey = (core * T + tl) * 2 + blk
    order = np.lexsort((row_in_blk, gkey))
    gkey_s = gkey[order]
    rows_s = row_in_blk[order]
    dcol_s = dcol[order]
    vals_s = vals[order]
    n_groups = C * T * 2
    bounds = np.searchsorted(gkey_s, np.arange(n_groups + 1))

    uniq = [None] * n_groups
    invs = [None] * n_groups
    for g in range(n_groups):
        lo, hi = bounds[g], bounds[g + 1]
        r = rows_s[lo:hi]
        u = np.unique(r)
        uniq[g] = u
        invs[g] = np.searchsorted(u, r)

    # SPMD-uniform schedule: max over cores per (tile, half)
    nuniq = np.zeros((C, T, 2), np.int64)
    for g in range(n_groups):
        c, rem = divmod(g, T * 2)
        t, h = divmod(rem, 2)
        nuniq[c, t, h] = len(uniq[g])
    reg = np.maximum(nuniq.max(axis=0), 1)  # [T, 2]
    sched = (reg + P - 1) // P              # [T, 2] chunks (128-multiples)
    SCH = int(sched.sum())

    # chunk layout: per tile, half A then half B
    chunk_off = np.zeros((T, 2), np.int64)
    acc = 0
    for t in range(T):
        for h in range(2):
            chunk_off[t, h] = acc
            acc += int(sched[t, h])
    assert acc == SCH

    # global dinv-scaled x table in block layout (replicated to all cores)
    xs = (dinv[:, None] * x).astype(np.float32)
    xpad = np.zeros((C, NPAD, P), np.float32)
    for c in range(C):
        xpad[c, :NPC] = xs[c * NPC : (c + 1) * NPC]
    xA = np.ascontiguousarray(xpad[:, :BLKA].reshape(FULLA, P)).astype(ml_dtypes.bfloat16)
    xB = np.ascontiguousarray(xpad[:, BLKA:].reshape(FULLB, P)).astype(ml_dtypes.bfloat16)

    common = {
        "xA": xA, "xB": xB,
        "w1": W1.astype(ml_dtypes.bfloat16),
        "w2": W2.astype(ml_dtypes.bfloat16),
        "lin1": np.ascontiguousarray(lin_W[:D]).astype(ml_dtypes.bfloat16),
        "lin2": np.ascontiguousarray(lin_W[D:]).astype(ml_dtypes.bfloat16),
        "ident": np.eye(P, dtype=np.float32).astype(ml_dtypes.bfloat16),
        "b1": b1[:, None].astype(np.float32),
        "b2": b2[:, None].astype(np.float32),
        "linb": np.tile(lin_b, (P, 1)).astype(np.float32),
    }

    in_maps = []
    for c in range(C):
        idx_cols = np.full((P, SCH * 8), -1, np.int16)
        cap = E // C + 4 * SCH * P + E // 16
        flat_rows = np.zeros(cap, np.int64)
        flat_dcol = np.zeros(cap, np.int64)
        flat_vals = np.zeros(cap, np.float32)
        n_e = 0
        for t in range(T):
            for h in range(2):
                g = (c * T + t) * 2 + h
                nch = int(sched[t, h])
                K = nch * P
                u = uniq[g]
                u_pad = np.zeros(K, np.int64)
                u_pad[: len(u)] = u
                wrapped = u_pad.astype(np.int16).reshape(nch * 8, 16).T
                co = int(chunk_off[t, h])
                idx_cols[:, co * 8 : (co + nch) * 8] = np.tile(wrapped, (8, 1))
                lo, hi = bounds[g], bounds[g + 1]
                ne = hi - lo
                flat_rows[n_e : n_e + ne] = co * P + invs[g]
                flat_dcol[n_e : n_e + ne] = dcol_s[lo:hi]
                flat_vals[n_e : n_e + ne] = vals_s[lo:hi]
                n_e += ne
        flat = flat_rows[:n_e] * P + flat_dcol[:n_e]
        s_core = np.bincount(flat, weights=flat_vals[:n_e], minlength=SCH * P * P)
        s_core = s_core.astype(np.float32).reshape(SCH, P, P)
        smat = np.ascontiguousarray(s_core.transpose(1, 0, 2)).reshape(P, SCH * P)

        # local x~ tiles, row-major [node-in-tile (part), tile, feat]
        xloc = np.ascontiguousarray(
            xpad[c].reshape(T, P, P).transpose(1, 0, 2)
        ).reshape(P, T * P)

        dv = np.zeros(NPAD, np.float32)
        dv[:NPC] = dinv[c * NPC : (c + 1) * NPC]
        dinv_tiles = np.ascontiguousarray(dv.reshape(T, P).T)  # [P, T]

        in_maps.append(dict(common) | {
            "xloc": xloc.astype(ml_dtypes.bfloat16),
            "dinv": dinv_tiles,
            "idx": idx_cols,
            "smat": smat.astype(ml_dtypes.bfloat16),
        })

    plan = {
        "N": N, "D": D, "E": E, "C": C, "NPC": NPC, "NPAD": NPAD, "T": T,
        "TA": TA, "TB": TB, "BLKA": BLKA, "BLKB": BLKB,
        "FULLA": FULLA, "FULLB": FULLB, "SCH": SCH,
        "sched": sched, "chunk_off": chunk_off,
    }
    return plan, in_maps


def _build(plan):
    T, TA, TB = plan["T"], plan["TA"], plan["TB"]
    NPAD = plan["NPAD"]
    BLKA, BLKB = plan["BLKA"], plan["BLKB"]
    FULLA, FULLB = plan["FULLA"], plan["FULLB"]
    SCH = plan["SCH"]
    sched = plan["sched"]
    chunk_off = plan["chunk_off"]
    maxch = int(sched.max())

    nc = bacc.Bacc(
        get_trn_type() or "TRN2",
        target_bir_lowering=False,
        debug=False,
        num_devices=N_CORES,
    )
    xA_in = nc.dram_tensor("xA", [FULLA, P], bf16, kind="ExternalInput").ap()
    xB_in = nc.dram_tensor("xB", [FULLB, P], bf16, kind="ExternalInput").ap()
    xloc_in = nc.dram_tensor("xloc", [P, T * P], bf16, kind="ExternalInput").ap()
    w1_in = nc.dram_tensor("w1", [P, P], bf16, kind="ExternalInput").ap()
    w2_in = nc.dram_tensor("w2", [P, P], bf16, kind="ExternalInput").ap()
    lin1_in = nc.dram_tensor("lin1", [P, P], bf16, kind="ExternalInput").ap()
    lin2_in = nc.dram_tensor("lin2", [P, P], bf16, kind="ExternalInput").ap()
    b1_in = nc.dram_tensor("b1", [P, 1], f32, kind="ExternalInput").ap()
    b2_in = nc.dram_tensor("b2", [P, 1], f32, kind="ExternalInput").ap()
    linb_in = nc.dram_tensor("linb", [P, P], f32, kind="ExternalInput").ap()
    dinv_in = nc.dram_tensor("dinv", [P, T], f32, kind="ExternalInput").ap()
    idx_in = nc.dram_tensor("idx", [P, SCH * 8], i16, kind="ExternalInput").ap()
    ident_in = nc.dram_tensor("ident", [P, P], bf16, kind="ExternalInput").ap()
    smat_in = nc.dram_tensor("smat", [P, SCH * P], bf16, kind="ExternalInput").ap()
    out_ap = nc.dram_tensor("out", [NPAD, P], f32, kind="ExternalOutput").ap()
    out_v = out_ap.rearrange("(t p) f -> p t f", p=P)

    nc.gpsimd.load_library(mlp)

    with tile.TileContext(nc) as tc:
        with (
            tc.tile_pool(name="dram", bufs=1, space="DRAM") as dram,
            tc.tile_pool(name="consts", bufs=1) as consts,
            tc.tile_pool(name="acts", bufs=1) as acts,
            tc.tile_pool(name="msg", bufs=6) as msgp,
            tc.tile_pool(name="smatp", bufs=6) as smatp,
            tc.tile_pool(name="diag", bufs=3) as diagp,
            tc.tile_pool(name="ysb", bufs=3) as ysbp,
            tc.tile_pool(name="otile", bufs=3) as otilep,
            tc.tile_pool(name="ps_y", bufs=3, space="PSUM") as psy,
            tc.tile_pool(name="ps_tf", bufs=4, space="PSUM") as pstf,
        ):
            w1 = consts.tile([P, P], bf16, tag="w1")
            nc.sync.dma_start(w1[:], w1_in[:])
            w2 = consts.tile([P, P], bf16, tag="w2")
            nc.sync.dma_start(w2[:], w2_in[:])
            lin1 = consts.tile([P, P], bf16, tag="lin1")
            nc.sync.dma_start(lin1[:], lin1_in[:])
            lin2 = consts.tile([P, P], bf16, tag="lin2")
            nc.sync.dma_start(lin2[:], lin2_in[:])
            b1 = consts.tile([P, 1], f32, tag="b1")
            nc.sync.dma_start(b1[:], b1_in[:])
            b2 = consts.tile([P, 1], f32, tag="b2")
            nc.sync.dma_start(b2[:], b2_in[:])
            linb = consts.tile([P, P], f32, tag="linb")
            nc.sync.dma_start(linb[:], linb_in[:])
            dinv = consts.tile([P, T], f32, tag="dinv")
            nc.sync.dma_start(dinv[:], dinv_in[:])
            idx_sb = consts.tile([P, SCH * 8], i16, tag="idx")
            idx_head = int(chunk_off[2, 0]) * 8 if T > 2 else SCH * 8
            nc.sync.dma_start(idx_sb[:, 0:idx_head], idx_in[:, 0:idx_head])
            nc.sync.dma_start(idx_sb[:, idx_head:], idx_in[:, idx_head:])
            ident = consts.tile([P, P], bf16, tag="ident")
            nc.sync.dma_start(ident[:], ident_in[:])

            xloc = acts.tile([P, T * P], bf16, tag="xloc")
            nc.sync.dma_start(xloc[:], xloc_in[:])
            x1loc = acts.tile([P, T * P], bf16, tag="x1loc")   # dinv-scaled x1
            x1T = acts.tile([P, NPAD], bf16, tag="x1T")        # feat-major
            x2T = acts.tile([P, NPAD], bf16, tag="x2T")
            partial = acts.tile([P, NPAD], f32, tag="partial")

            # memset msg buffers once so trailing-trimmed (skipped) gather
            # chunks read as finite values (x * 0 = 0 in the S matmul)
            for _ in range(6):
                m0 = msgp.tile([P, maxch, P], bf16, tag="msg", name="msg")
                nc.vector.memset(m0[:, :, :], 0)

            g_loc = [None, None]
            g_full = [None, None]
            for h, (blkrows, fullrows) in enumerate([(BLKA, FULLA), (BLKB, FULLB)]):
                g_loc[h] = dram.tile([blkrows, P], bf16, tag=f"gloc{h}",
                                     name=f"gloc{h}")
                g_full[h] = dram.tile([fullrows, P], bf16, tag=f"gfull{h}",
                                      name=f"gfull{h}")

            tabA = [xA_in, g_full[0]]
            tabB = [xB_in, g_full[1]]

            def gather_th(layer, t, h):
                """Issue dma_gather + S load for (tile t, half h); return
                (msg, s_sb, nch)."""
                nch = int(sched[t, h])
                K = nch * P
                co = int(chunk_off[t, h])
                msg = msgp.tile([P, maxch, P], bf16, tag="msg", name="msg")
                s_sb = smatp.tile([P, maxch * P], bf16, tag="smat", name="ssb")
                nc.sync.dma_start(
                    s_sb[:, 0 : nch * P], smat_in[:, co * P : (co + nch) * P]
                )
                table = tabA[layer] if h == 0 else tabB[layer]
                nc.gpsimd.dma_gather(
                    msg[:, 0:nch, :], table[:], idx_sb[:, co * 8 : (co + nch) * 8],
                    K, K, P, single_packet=False,
                )
                return msg, s_sb, nch

            def agg_tile(ps, pieces, t, src_loc, add_partial):
                """S-matmul chain + self-loop (+partial) into psum ps."""
                first = True
                for msg, s_sb, nch in pieces:
                    for j in range(nch):
                        nc.tensor.matmul(
                            ps[:], lhsT=msg[:, j, :], rhs=s_sb[:, bass.ts(j, P)],
                            start=first, stop=False,
                        )
                        first = False
                diag = diagp.tile([P, P], bf16, tag="diag", name="diag")
                nc.vector.tensor_scalar(
                    diag[:], ident[:], dinv[:, t : t + 1], None,
                    mybir.AluOpType.mult,
                )
                nc.tensor.matmul(
                    ps[:], lhsT=src_loc[:, bass.ts(t, P)], rhs=diag[:],
                    start=first, stop=True,
                )
                if add_partial:
                    nc.vector.tensor_tensor(
                        out=ps[:], in0=ps[:], in1=partial[:, bass.ts(t, P)],
                        op=mybir.AluOpType.add,
                    )

            def transform_tile(ps, t, w_tile, bias, xT_out, make_table):
                """Y(psum) -> xT (feat-major) and optionally the dinv-scaled
                row-major table tile + DRAM write."""
                ysb = ysbp.tile([P, P], bf16, tag="ysb", name="ysb")
                nc.vector.tensor_copy(out=ysb[:], in_=ps[:])
                ps2 = pstf.tile([P, P], f32, tag="ps_tf", name="pstf")
                nc.tensor.matmul(ps2[:], lhsT=w_tile[:], rhs=ysb[:],
                                 start=True, stop=True)
                nc.scalar.activation(
                    xT_out[:, bass.ts(t, P)], ps2[:],
                    mybir.ActivationFunctionType.Relu, bias=bias[:],
                )
                if make_table:
                    psT = pstf.tile([P, P], f32, tag="ps_tf", name="pstf")
                    nc.tensor.matmul(psT[:], lhsT=xT_out[:, bass.ts(t, P)],
                                     rhs=ident[:], start=True, stop=True)
                    nc.vector.tensor_scalar(
                        x1loc[:, bass.ts(t, P)], psT[:],
                        dinv[:, t : t + 1], None, mybir.AluOpType.mult,
                    )

            def allgather(h):
                lo = 0 if h == 0 else BLKA
                hi = BLKA if h == 0 else NPAD
                nc.sync.dma_start(
                    g_loc[h][:].rearrange("(tt p) f -> p tt f", p=P),
                    x1loc[:, lo:hi].rearrange("p (tt f) -> p tt f", f=P),
                )
                nc.gpsimd.collective_compute(
                    "AllGather",
                    mybir.AluOpType.bypass,
                    replica_groups=[list(range(N_CORES))],
                    ins=[g_loc[h].opt()],
                    outs=[g_full[h].opt()],
                )

            # ---------------- layer 1: single pass over tiles ----------------
            for t in range(T):
                pieces = [gather_th(0, t, 0), gather_th(0, t, 1)]
                ps = psy.tile([P, P], f32, tag="ps_y", name="psy")
                agg_tile(ps, pieces, t, xloc, add_partial=False)
                transform_tile(ps, t, w1, b1, x1T, make_table=True)
                if t == TA:  # tiles 0..TA-1 (A rows) written -> start AllGather
                    allgather(0)
            allgather(1)

            # ---------------- layer 2: pass 1 (A half) -> partial ------------
            for t in range(T):
                msg, s_sb, nch = gather_th(1, t, 0)
                ps = psy.tile([P, P], f32, tag="ps_y", name="psy")
                for j in range(nch):
                    nc.tensor.matmul(
                        ps[:], lhsT=msg[:, j, :], rhs=s_sb[:, bass.ts(j, P)],
                        start=(j == 0), stop=(j == nch - 1),
                    )
                nc.vector.tensor_copy(out=partial[:, bass.ts(t, P)], in_=ps[:])

            # ---------------- layer 2: pass 2 (B half) + final ---------------
            for t in range(T):
                pieces = [gather_th(1, t, 1)]
                ps = psy.tile([P, P], f32, tag="ps_y", name="psy")
                agg_tile(ps, pieces, t, x1loc, add_partial=True)
                transform_tile(ps, t, w2, b2, x2T, make_table=False)
                # final: out_t = x1_t @ lin1 + x2_t @ lin2 + lin_b
                ps3 = pstf.tile([P, P], f32, tag="ps_tf", name="pstf")
                nc.tensor.matmul(ps3[:], lhsT=x1T[:, bass.ts(t, P)],
                                 rhs=lin1[:], start=True, stop=False)
                nc.tensor.matmul(ps3[:], lhsT=x2T[:, bass.ts(t, P)],
                                 rhs=lin2[:], start=False, stop=True)
                ot = otilep.tile([P, P], f32, tag="otile", name="otile")
                nc.vector.tensor_tensor(
                    out=ot[:], in0=ps3[:], in1=linb[:], op=mybir.AluOpType.add
                )
                nc.sync.dma_start(out_v[:, t, :], ot[:])

    nc.compile()
    return nc


def kernel(x, edge_index, W1, b1, W2, b2, lin_W, lin_b):
    x = np.asarray(x, np.float32)
    edge_index = np.asarray(edge_index)
    W1 = np.asarray(W1, np.float32)
    W2 = np.asarray(W2, np.float32)
    b1 = np.asarray(b1, np.float32)
    b2 = np.asarray(b2, np.float32)
    lin_W = np.asarray(lin_W, np.float32)
    lin_b = np.asarray(lin_b, np.float32)

    plan, in_maps = _preprocess(x, edge_index, W1, b1, W2, b2, lin_W, lin_b)
    nc = _build(plan)

    N, D, C, NPC = plan["N"], plan["D"], plan["C"], plan["NPC"]
    last_err = None
    for _attempt in range(3):
        try:
            res = run_bass_kernel_spmd(nc, in_maps, list(range(C)))
            break
        except Exception as e:  # transient NRT device wedges happen
            last_err = e
    else:
        raise last_err

    out = np.empty((N, D), np.float32)
    for c in range(C):
        out[c * NPC : (c + 1) * NPC] = res.results[c]["out"][:NPC]
    return out


# revision 13
# speedup vs baseline: 1.0649x; 1.0617x over previous
"""2-layer GCN + JumpingKnowledge(cat) + Linear on 8 Trainium2 NeuronCores.

v4 strategy (aggregate-first, bf16, continuous gather streams):
  - GCN identity  relu(D^-1/2 (A+I) D^-1/2 X W + b) = relu(((A+I)_norm X) W + b):
    aggregate FIRST, transform after.  Layer 1 gathers straight from a
    host-precomputed table  x~ = dinv * x  (bf16, replicated input): no
    phase matmul, no AllGather before the first gather.
  - Aggregation: dma_gather pulls unique source rows (bf16, 256B) from the
    table; TensorE matmul chain psum[feat,dst] += M_chunk^T @ S_chunk with
    S (host-built, bf16) carrying dinv[dst] and edge multiplicity.
    Self-loops via diag(dinv) against local row-major x~ tiles (no gather).
  - Continuous chunk stream per half-table: per destination tile the
    stream reserves exactly max-over-cores unique rows (no per-tile
    rounding to 128); 128-row chunks that straddle a tile boundary are
    matmul'd into both tiles' psums with separate S blocks.  This cuts
    schedule padding ~6% vs per-tile ceil.
  - Per layer: pass A (A-half stream -> partial), pass B (B-half stream +
    self-loop + partial + transform).  Layer-1 tiles also produce the
    dinv-scaled row-major table x~1 (PE transpose) which is AllGather'd
    (A-half triggered as soon as tiles 0..TA-1 finish, B at layer end);
    both collectives hide under the gather stream.
  - Final: out = x1 @ lin_W[:D] + x2 @ lin_W[D:] + lin_b per tile, fused
    into layer-2's pass B.

dma_gather needs int16 indices (<32768 rows), so tables are split in two
blocks: each core contributes rows [0,3200) to table A and [3200,6272) to
table B.
"""
import numpy as np
import ml_dtypes

import concourse.bass as bass
import concourse.bacc as bacc
import concourse.mybir as mybir
import concourse.tile as tile
from concourse._compat import get_trn_type
from concourse.bass_utils import run_bass_kernel_spmd
from concourse.library_config import mlp

P = 128
N_CORES = 8
GCH = 9  # chunks per gather instruction


def _halfplan(cap):
    """Fixed chunk-stream layout for one half: offsets, chunk->block map,
    gather grouping.  cap[t] = max-over-cores unique rows of tile t."""
    T = len(cap)
    O = np.zeros(T + 1, np.int64)
    O[1:] = np.cumsum(cap)
    CH = int((O[T] + P - 1) // P)
    # blocks in chunk order; chunk_blocks[c] = [(tile, block_id, is_last)]
    chunk_blocks = [[] for _ in range(CH)]
    nblk = 0
    block_of = {}
    for c in range(CH):
        lo, hi = c * P, (c + 1) * P
        for t in range(T):
            if O[t] < hi and O[t + 1] > lo:
                last = O[t + 1] <= hi
                chunk_blocks[c].append((t, nblk, last))
                block_of[(c, t)] = nblk
                nblk += 1
    gathers = []
    c = 0
    while c < CH:
        nch = min(GCH, CH - c)
        bk0 = chunk_blocks[c][0][1]
        bk1 = chunk_blocks[c + nch - 1][-1][1] + 1
        gathers.append((c, nch, bk0, bk1 - bk0))
        c += nch
    return {"O": O, "CH": CH, "chunk_blocks": chunk_blocks, "nblk": nblk,
            "block_of": block_of, "gathers": gathers}


def _preprocess(x, edge_index, W1, b1, W2, b2, lin_W, lin_b):
    """Host-side: shard, block-split, dedup, continuous-stream gather
    indices, S blocks, the dinv-scaled bf16 x table, and all in_maps."""
    N, D = x.shape
    assert D == P
    E = edge_index.shape[1]
    C = N_CORES
    NPC = (N + C - 1) // C
    assert NPC * C == N, "node count must split evenly across cores"
    NPAD = ((NPC + P - 1) // P) * P
    T = NPAD // P
    TA = (T + 1) // 2
    TB = T - TA
    BLKA, BLKB = TA * P, TB * P
    FULLA, FULLB = C * BLKA, C * BLKB
    assert FULLA <= 32768 and FULLB <= 32768

    src = edge_index[0].astype(np.int64)
    dst = edge_index[1].astype(np.int64)

    deg = np.bincount(dst, minlength=N).astype(np.float32) + 1.0
    dinv = (1.0 / np.sqrt(deg)).astype(np.float32)

    off = src % NPC
    blk = (off >= BLKA).astype(np.int64)
    row_in_blk = np.where(blk == 0, (src // NPC) * BLKA + off,
                          (src // NPC) * BLKB + (off - BLKA))
    core = dst // NPC
    dloc = dst % NPC
    tl = dloc // P
    dcol = dloc % P
    vals = dinv[dst]

    gkey = (core * T + tl) * 2 + blk
    order = np.lexsort((row_in_blk, gkey))
    gkey_s = gkey[order]
    rows_s = row_in_blk[order]
    dcol_s = dcol[order]
    vals_s = vals[order]
    n_groups = C * T * 2
    bounds = np.searchsorted(gkey_s, np.arange(n_groups + 1))

    uniq = [None] * n_groups
    invs = [None] * n_groups
    nuniq = np.zeros((C, T, 2), np.int64)
    for g in range(n_groups):
        lo, hi = bounds[g], bounds[g + 1]
        r = rows_s[lo:hi]
        u = np.unique(r)
        uniq[g] = u
        invs[g] = np.searchsorted(u, r)
        c, rem = divmod(g, T * 2)
        t, h = divmod(rem, 2)
        nuniq[c, t, h] = len(u)

    cap = np.maximum(nuniq.max(axis=0), 1)  # [T, 2]
    hp = [_halfplan(cap[:, 0]), _halfplan(cap[:, 1])]
    CH_A, CH_B = hp[0]["CH"], hp[1]["CH"]
    CHT = CH_A + CH_B
    NBLK = hp[0]["nblk"] + hp[1]["nblk"]

    # global dinv-scaled x table in block layout (replicated to all cores)
    xs = (dinv[:, None] * x).astype(np.float32)
    xpad = np.zeros((C, NPAD, P), np.float32)
    for c in range(C):
        xpad[c, :NPC] = xs[c * NPC : (c + 1) * NPC]
    xA = np.ascontiguousarray(xpad[:, :BLKA].reshape(FULLA, P)).astype(ml_dtypes.bfloat16)
    xB = np.ascontiguousarray(xpad[:, BLKA:].reshape(FULLB, P)).astype(ml_dtypes.bfloat16)

    common = {
        "xA": xA, "xB": xB,
        "w1": W1.astype(ml_dtypes.bfloat16),
        "w2": W2.astype(ml_dtypes.bfloat16),
        "lin1": np.ascontiguousarray(lin_W[:D]).astype(ml_dtypes.bfloat16),
        "lin2": np.ascontiguousarray(lin_W[D:]).astype(ml_dtypes.bfloat16),
        "ident": np.eye(P, dtype=np.float32).astype(ml_dtypes.bfloat16),
        "b1": b1[:, None].astype(np.float32),
        "b2": b2[:, None].astype(np.float32),
        "linb": np.tile(lin_b, (P, 1)).astype(np.float32),
    }

    in_maps = []
    for c in range(C):
        idx_cols = np.zeros((P, CHT * 8), np.int16)
        sidx = np.zeros(NBLK * P * P, np.float32)  # dense S, [block, row, dcol]
        for h in range(2):
            plan_h = hp[h]
            O = plan_h["O"]
            base_c = 0 if h == 0 else CH_A
            base_b = 0 if h == 0 else hp[0]["nblk"]
            stream = np.zeros(plan_h["CH"] * P, np.int64)
            for t in range(T):
                g = (c * T + t) * 2 + h
                u = uniq[g]
                stream[O[t] : O[t] + len(u)] = u
            w = stream.astype(np.int16).reshape(plan_h["CH"] * 8, 16).T
            idx_cols[:, base_c * 8 : (base_c + plan_h["CH"]) * 8] = np.tile(w, (8, 1))
            # S entries: edge (pos, dcol, val); pos = O[t] + inv
            for t in range(T):
                g = (c * T + t) * 2 + h
                lo, hi = bounds[g], bounds[g + 1]
                if hi == lo:
                    continue
                pos = O[t] + invs[g]
                ch = pos // P
                bk = np.array([base_b + plan_h["block_of"][(cc, t)] for cc in ch])
                flat = (bk * P + pos % P) * P + dcol_s[lo:hi]
                np.add.at(sidx, flat, vals_s[lo:hi])
        smat = np.ascontiguousarray(
            sidx.reshape(NBLK, P, P).transpose(1, 0, 2)
        ).reshape(P, NBLK * P)

        # local x~ tiles, row-major [node-in-tile (part), tile, feat]
        xloc = np.ascontiguousarray(
            xpad[c].reshape(T, P, P).transpose(1, 0, 2)
        ).reshape(P, T * P)

        dv = np.zeros(NPAD, np.float32)
        dv[:NPC] = dinv[c * NPC : (c + 1) * NPC]
        dinv_tiles = np.ascontiguousarray(dv.reshape(T, P).T)  # [P, T]

        in_maps.append(dict(common) | {
            "xloc": xloc.astype(ml_dtypes.bfloat16),
            "dinv": dinv_tiles,
            "idx": idx_cols,
            "smat": sidx.reshape(NBLK, P, P).transpose(1, 0, 2).reshape(
                P, NBLK * P).astype(ml_dtypes.bfloat16),
        })

    plan = {
        "N": N, "D": D, "E": E, "C": C, "NPC": NPC, "NPAD": NPAD, "T": T,
        "TA": TA, "TB": TB, "BLKA": BLKA, "BLKB": BLKB,
        "FULLA": FULLA, "FULLB": FULLB,
        "CH_A": CH_A, "CH_B": CH_B, "CHT": CHT, "NBLK": NBLK, "hp": hp,
    }
    return plan, in_maps


def _build(plan):
    T, TA = plan["T"], plan["TA"]
    NPAD = plan["NPAD"]
    BLKA = plan["BLKA"]
    FULLA, FULLB = plan["FULLA"], plan["FULLB"]
    CH_A, CHT, NBLK = plan["CH_A"], plan["CHT"], plan["NBLK"]
    hp = plan["hp"]

    f32 = mybir.dt.float32
    bf16 = mybir.dt.bfloat16
    i16 = mybir.dt.int16

    nc = bacc.Bacc(
        get_trn_type() or "TRN2",
        target_bir_lowering=False,
        debug=False,
        num_devices=N_CORES,
    )
    xA_in = nc.dram_tensor("xA", [FULLA, P], bf16, kind="ExternalInput").ap()
    xB_in = nc.dram_tensor("xB", [FULLB, P], bf16, kind="ExternalInput").ap()
    xloc_in = nc.dram_tensor("xloc", [P, T * P], bf16, kind="ExternalInput").ap()
    w1_in = nc.dram_tensor("w1", [P, P], bf16, kind="ExternalInput").ap()
    w2_in = nc.dram_tensor("w2", [P, P], bf16, kind="ExternalInput").ap()
    lin1_in = nc.dram_tensor("lin1", [P, P], bf16, kind="ExternalInput").ap()
    lin2_in = nc.dram_tensor("lin2", [P, P], bf16, kind="ExternalInput").ap()
    b1_in = nc.dram_tensor("b1", [P, 1], f32, kind="ExternalInput").ap()
    b2_in = nc.dram_tensor("b2", [P, 1], f32, kind="ExternalInput").ap()
    linb_in = nc.dram_tensor("linb", [P, P], f32, kind="ExternalInput").ap()
    dinv_in = nc.dram_tensor("dinv", [P, T], f32, kind="ExternalInput").ap()
    idx_in = nc.dram_tensor("idx", [P, CHT * 8], i16, kind="ExternalInput").ap()
    ident_in = nc.dram_tensor("ident", [P, P], bf16, kind="ExternalInput").ap()
    smat_in = nc.dram_tensor("smat", [P, NBLK * P], bf16, kind="ExternalInput").ap()
    out_ap = nc.dram_tensor("out", [NPAD, P], f32, kind="ExternalOutput").ap()
    out_v = out_ap.rearrange("(t p) f -> p t f", p=P)

    nc.gpsimd.load_library(mlp)

    with tile.TileContext(nc) as tc:
        with (
            tc.tile_pool(name="dram", bufs=1, space="DRAM") as dram,
            tc.tile_pool(name="consts", bufs=1) as consts,
            tc.tile_pool(name="acts", bufs=1) as acts,
            tc.tile_pool(name="msg", bufs=6) as msgp,
            tc.tile_pool(name="smatp", bufs=6) as smatp,
            tc.tile_pool(name="diag", bufs=3) as diagp,
            tc.tile_pool(name="ysb", bufs=3) as ysbp,
            tc.tile_pool(name="otile", bufs=3) as otilep,
            tc.tile_pool(name="ps_y", bufs=3, space="PSUM") as psy,
            tc.tile_pool(name="ps_tf", bufs=4, space="PSUM") as pstf,
        ):
            w1 = consts.tile([P, P], bf16, tag="w1")
            nc.sync.dma_start(w1[:], w1_in[:])
            w2 = consts.tile([P, P], bf16, tag="w2")
            nc.sync.dma_start(w2[:], w2_in[:])
            lin1 = consts.tile([P, P], bf16, tag="lin1")
            nc.sync.dma_start(lin1[:], lin1_in[:])
            lin2 = consts.tile([P, P], bf16, tag="lin2")
            nc.sync.dma_start(lin2[:], lin2_in[:])
            b1 = consts.tile([P, 1], f32, tag="b1")
            nc.sync.dma_start(b1[:], b1_in[:])
            b2 = consts.tile([P, 1], f32, tag="b2")
            nc.sync.dma_start(b2[:], b2_in[:])
            linb = consts.tile([P, P], f32, tag="linb")
            nc.sync.dma_start(linb[:], linb_in[:])
            dinv = consts.tile([P, T], f32, tag="dinv")
            nc.sync.dma_start(dinv[:], dinv_in[:])
            idx_sb = consts.tile([P, CHT * 8], i16, tag="idx")
            idx_head = GCH * 8
            nc.sync.dma_start(idx_sb[:, 0:idx_head], idx_in[:, 0:idx_head])
            nc.sync.dma_start(idx_sb[:, idx_head:], idx_in[:, idx_head:])
            ident = consts.tile([P, P], bf16, tag="ident")
            nc.sync.dma_start(ident[:], ident_in[:])

            xloc = acts.tile([P, T * P], bf16, tag="xloc")
            nc.sync.dma_start(xloc[:], xloc_in[:])
            x1loc = acts.tile([P, T * P], bf16, tag="x1loc")
            x1T = acts.tile([P, NPAD], bf16, tag="x1T")
            x2T = acts.tile([P, NPAD], bf16, tag="x2T")
            partial = acts.tile([P, NPAD], f32, tag="partial")

            g_loc = [None, None]
            g_full = [None, None]
            for h, (blkrows, fullrows) in enumerate(
                [(BLKA, FULLA), (NPAD - BLKA, FULLB)]
            ):
                g_loc[h] = dram.tile([blkrows, P], bf16, tag=f"gloc{h}",
                                     name=f"gloc{h}")
                g_full[h] = dram.tile([fullrows, P], bf16, tag=f"gfull{h}",
                                      name=f"gfull{h}")

            tabs = [[xA_in, xB_in], [g_full[0], g_full[1]]]

            def transform_tile(ps, t, w_tile, bias, xT_out, make_table):
                ysb = ysbp.tile([P, P], bf16, tag="ysb", name="ysb")
                nc.vector.tensor_copy(out=ysb[:], in_=ps[:])
                ps2 = pstf.tile([P, P], f32, tag="ps_tf", name="pstf")
                nc.tensor.matmul(ps2[:], lhsT=w_tile[:], rhs=ysb[:],
                                 start=True, stop=True)
                nc.scalar.activation(
                    xT_out[:, bass.ts(t, P)], ps2[:],
                    mybir.ActivationFunctionType.Relu, bias=bias[:],
                )
                if make_table:
                    psT = pstf.tile([P, P], f32, tag="ps_tf", name="pstf")
                    nc.tensor.matmul(psT[:], lhsT=xT_out[:, bass.ts(t, P)],
                                     rhs=ident[:], start=True, stop=True)
                    nc.vector.tensor_scalar(
                        x1loc[:, bass.ts(t, P)], psT[:],
                        dinv[:, t : t + 1], None, mybir.AluOpType.mult,
                    )

            def allgather(h):
                lo = 0 if h == 0 else BLKA
                hi = BLKA if h == 0 else NPAD
                nc.sync.dma_start(
                    g_loc[h][:].rearrange("(tt p) f -> p tt f", p=P),
                    x1loc[:, lo:hi].rearrange("p (tt f) -> p tt f", f=P),
                )
                nc.gpsimd.collective_compute(
                    "AllGather",
                    mybir.AluOpType.bypass,
                    replica_groups=[list(range(N_CORES))],
                    ins=[g_loc[h].opt()],
                    outs=[g_full[h].opt()],
                )

            def half_pass(layer, h, finalize, hooks=None, stop_on_last=False):
                """Stream the half's chunks; finalize(t, ps, first) when a
                tile's last chunk has been matmul'd (pass decides stop)."""
                plan_h = hp[h]
                base_c = 0 if h == 0 else CH_A
                base_b = 0 if h == 0 else hp[0]["nblk"]
                table = tabs[layer][h]
                open_ps = {}
                for (cs, nch, bk0, nbk) in plan_h["gathers"]:
                    msg = msgp.tile([P, GCH, P], bf16, tag="msg", name="msg")
                    s_sb = smatp.tile([P, (GCH + 3) * P], bf16, tag="smat",
                                      name="ssb")
                    nc.sync.dma_start(
                        s_sb[:, 0 : nbk * P],
                        smat_in[:, (base_b + bk0) * P : (base_b + bk0 + nbk) * P],
                    )
                    nc.gpsimd.dma_gather(
                        msg[:, 0:nch, :], table[:],
                        idx_sb[:, (base_c + cs) * 8 : (base_c + cs + nch) * 8],
                        nch * P, nch * P, P, single_packet=False,
                    )
                    for ci in range(nch):
                        for (t, bk, last) in plan_h["chunk_blocks"][cs + ci]:
                            first = t not in open_ps
                            if first:
                                open_ps[t] = psy.tile([P, P], f32, tag="ps_y",
                                                      name="psy")
                            ps = open_ps[t]
                            nc.tensor.matmul(
                                ps[:], lhsT=msg[:, ci, :],
                                rhs=s_sb[:, bass.ts(bk - bk0, P)],
                                start=first, stop=(stop_on_last and last),
                            )
                            if last:
                                finalize(t, open_ps.pop(t))
                                if hooks and t in hooks:
                                    hooks[t]()
                assert not open_ps

            # ---------------- layer passes ----------------
            def run_layer(layer, w_tile, bias, xT_out, make_table,
                          src_loc, hooks=None):
                # pass A -> partial
                def finA(t, ps):
                    nc.vector.tensor_copy(out=partial[:, bass.ts(t, P)],
                                          in_=ps[:])
                half_pass(layer, 0, finA, stop_on_last=True)

                # pass B: + self-loop + partial + transform
                def finB(t, ps):
                    diag = diagp.tile([P, P], bf16, tag="diag", name="diag")
                    nc.vector.tensor_scalar(
                        diag[:], ident[:], dinv[:, t : t + 1], None,
                        mybir.AluOpType.mult,
                    )
                    nc.tensor.matmul(
                        ps[:], lhsT=src_loc[:, bass.ts(t, P)], rhs=diag[:],
                        start=False, stop=True,
                    )
                    nc.vector.tensor_tensor(
                        out=ps[:], in0=ps[:], in1=partial[:, bass.ts(t, P)],
                        op=mybir.AluOpType.add,
                    )
                    transform_tile(ps, t, w_tile, bias, xT_out, make_table)
                    if layer == 1:
                        ps3 = pstf.tile([P, P], f32, tag="ps_tf", name="pstf")
                        nc.tensor.matmul(ps3[:], lhsT=x1T[:, bass.ts(t, P)],
                                         rhs=lin1[:], start=True, stop=False)
                        nc.tensor.matmul(ps3[:], lhsT=x2T[:, bass.ts(t, P)],
                                         rhs=lin2[:], start=False, stop=True)
                        ot = otilep.tile([P, P], f32, tag="otile", name="otile")
                        nc.vector.tensor_tensor(
                            out=ot[:], in0=ps3[:], in1=linb[:],
                            op=mybir.AluOpType.add,
                        )
                        nc.sync.dma_start(out_v[:, t, :], ot[:])
                half_pass(layer, 1, finB, hooks=hooks)

            run_layer(0, w1, b1, x1T, True, xloc,
                      hooks={TA - 1: lambda: allgather(0),
                             T - 1: lambda: allgather(1)})
            run_layer(1, w2, b2, x2T, False, x1loc)

    nc.compile()
    return nc


def kernel(x, edge_index, W1, b1, W2, b2, lin_W, lin_b):
    x = np.asarray(x, np.float32)
    edge_index = np.asarray(edge_index)
    W1 = np.asarray(W1, np.float32)
    W2 = np.asarray(W2, np.float32)
    b1 = np.asarray(b1, np.float32)
    b2 = np.asarray(b2, np.float32)
    lin_W = np.asarray(lin_W, np.float32)
    lin_b = np.asarray(lin_b, np.float32)

    plan, in_maps = _preprocess(x, edge_index, W1, b1, W2, b2, lin_W, lin_b)
    nc = _build(plan)

    N, D, C, NPC = plan["N"], plan["D"], plan["C"], plan["NPC"]
    last_err = None
    for _attempt in range(3):
        try:
            res = run_bass_kernel_spmd(nc, in_maps, list(range(C)))
            break
        except Exception as e:  # transient NRT device wedges happen
            last_err = e
    else:
        raise last_err

    out = np.empty((N, D), np.float32)
    for c in range(C):
        out[c * NPC : (c + 1) * NPC] = res.results[c]["out"][:NPC]
    return out
